# revision 6
# baseline (speedup 1.0000x reference)
"""LIF spiking-neuron kernel for Trainium2 (Bass/Tile), 8-core SPMD.

Problem: x [B=32, T=8, C=128, H=32, W=32] f32.  Per (b,c,h,w) neuron,
sequential over T:
    mem = mem*TAU + x_t;  spike = (mem - 1 > 0);  mem = 0 if spike
TAU = 0.5, THRESH = 1.0.

Sharding: batch dim B=32 split 4-per-core across 8 NeuronCores; the
recurrence is along T only, so there is no communication.

Per-core algorithm (bit-exact vs the fp32 reference):
  TAU = 0.5 is a power of two, so rescale the state M_t = 2^t * m_t.
  The decay becomes a pure add:  M_t = M_{t-1} + 2^t * x_t  (the 2^t
  prescale of x is exact in fp32, and power-of-2 scaling commutes with
  fp rounding, so every M_t is bit-exactly 2^t * m_t).
  spike_t = (M_t > 2^t)  <=>  (m_t > 1)  <=>  reference's (m_t - 1 > 0).

  The whole state update (previous step's reset + prescale + accumulate)
  is ONE fused custom-DVE op per step:
      M_t = select(M_{t-1} > 2^(t-1), 0, M_{t-1}) + x_t * 2^t
  so the spike computation is only an output tap, off the recurrence
  critical path.  Spikes are written to DRAM as uint8 (4x less output
  DMA traffic than f32) and upcast to f32 on the host.

Engine split per (b,t) tile of [C=128 partitions, H*W*PAIR=2048]:
  DVE:  fused state update (14 ops/core) -- the only recurrence work
  ACT:  chain-0 spikes: Sign(M - 2^t) -> u8 (saturating cast maps -1 to
        0, verified on HW), plus chain-1 input DMA triggers and chain-0
        output DMA triggers
  GPS:  chain-1 spikes: (M > 2^t) -> u8 via tensor_single_scalar
  SP :  chain-0 input DMA triggers, chain-1 output DMA triggers
"""

import re

import numpy as np

from concourse import bacc, bass, mybir, tile
from concourse import dve_ops
from concourse.alu_op_type import AluOpType
from concourse.bass_utils import run_bass_kernel_spmd
from concourse.dve_spec import Spec, Src0, Src1, C0, C1, Zero, select

# Full-problem shape (hardcoded per harness contract).
B, T, C, H, W = 32, 8, 128, 32, 32
N_CORES = 8
B_LOC = B // N_CORES          # 4 batches per core
F = H * W                     # 1024 free elements per (b, t, c)
FP32 = mybir.dt.float32
U8 = mybir.dt.uint8

PAIR = 2                      # batches fused per tile
G = B_LOC // PAIR             # chain groups per core
FW = PAIR * F                 # 2048 free elements per tile

_NC_CACHE = {}


def _register_lif_op():
    """Register the fused LIF state-update custom-DVE op (idempotent).

    out = select(in1 > s1, 0, in1) + in0 * s0
        = (previous step's hard reset) + (2^t-prescaled input)
    """
    name = "LIF_STEP_ANT"
    if name in dve_ops._SUB_OPCODE_FOR_NAME:
        return next(op for op in dve_ops.OPS if op.name == name)
    body = select(Src1 > C1, Zero, Src1) + Src0 * C0
    op = dve_ops.DveOp(
        name,
        Spec(
            body=body,
            reference=lambda in0, in1, s0, s1, imm2: (
                np.where(in1 > s1, np.float32(0.0), in1) + in0 * s0
            ).astype(np.float32),
        ),
        subdim=False,
        uops_sha={},
    )
    dve_ops.OPS.append(op)
    dve_ops.CUSTOM_DVE_SPECS[name] = op.spec
    dve_ops._SUB_OPCODE_FOR_NAME[name] = (
        dve_ops._CUSTOM_DVE_ROW_BASE + len(dve_ops.OPS) - 1
    )
    # Pin uops_sha to whatever lower() produces in this process.
    for ver in ("v3", "v4"):
        try:
            op.compile(ver)
        except ValueError as e:
            m = re.search(r'"%s"\]="([0-9a-f]{16})"' % ver, str(e))
            if not m:
                raise
            op.uops_sha[ver] = m.group(1)
            dve_ops._COMPILE_CACHE.pop((name, ver), None)
            op.compile(ver)
    return op


LIF_STEP = _register_lif_op()


def _emit(tc, x_d, o_d):
    nc = tc.nc

    # per (group, t) DRAM view: [c, pair, h*w] -- 2 batches fused per tile
    def dram3(ap, g, t):
        return ap[g * PAIR : (g + 1) * PAIR, t].rearrange("p c h w -> c p (h w)")

    def as3(tile_ap):
        return tile_ap.rearrange("c (p f) -> c p f", p=PAIR)

    with (
        tc.tile_pool(name="xp", bufs=(T - 1) * G) as xp,
        tc.tile_pool(name="sp", bufs=6) as sp,
        tc.tile_pool(name="mp", bufs=6) as mp,
        tc.tile_pool(name="bp", bufs=1) as bp,
    ):
        # per-t [128,1] bias columns holding -2^t for the ACT Sign compare
        biases = []
        for t in range(T):
            bt = bp.tile([C, 1], FP32, name=f"bias{t}")
            nc.gpsimd.memset(bt, -float(2.0**t))
            biases.append(bt)

        # --- all input DMAs issued up front, t-major, SP/ACT split by chain.
        # t=0 lands directly in the chain's first membrane tile (M_0 = x_0).
        ms = {}
        for g in range(G):
            m0 = mp.tile([C, FW], FP32, name="mt")
            eng = nc.sync if g == 0 else nc.scalar
            eng.dma_start(out=as3(m0), in_=dram3(x_d, g, 0))
            ms[g] = m0
        xs = {}
        for t in range(1, T):
            for g in range(G):
                xt = xp.tile([C, FW], FP32)
                eng = nc.sync if g == 0 else nc.scalar
                eng.dma_start(out=as3(xt), in_=dram3(x_d, g, t))
                xs[(t, g)] = xt

        # --- recurrence (DVE) + spike taps (ACT / GPS) + u8 output DMAs
        for t in range(T):
            th = float(2.0**t)
            for g in range(G):
                if t > 0:
                    m_new = mp.tile([C, FW], FP32, name="mt")
                    nc.vector._custom_dve(
                        LIF_STEP,
                        out=m_new,
                        in0=xs[(t, g)],
                        in1=ms[g],
                        s0=th,
                        s1=th / 2.0,
                    )
                    ms[g] = m_new
                m = ms[g]
                s = sp.tile([C, FW], U8)
                if g == 0:
                    # ACT: u8 cast of Sign(M - 2^t) saturates -1 -> 0
                    nc.scalar.activation(
                        s, m, mybir.ActivationFunctionType.Sign, bias=biases[t]
                    )
                    nc.scalar.dma_start(out=dram3(o_d, g, t), in_=as3(s))
                else:
                    nc.gpsimd.tensor_single_scalar(s, m, th, AluOpType.is_gt)
                    nc.sync.dma_start(out=dram3(o_d, g, t), in_=as3(s))


def build_nc():
    """Build + compile the per-core Bass program (cached)."""
    if "nc" in _NC_CACHE:
        return _NC_CACHE["nc"]
    nc = bacc.Bacc(
        "TRN2",
        target_bir_lowering=False,
        debug=False,
        enable_asserts=False,
        num_devices=N_CORES,
    )
    x_d = nc.dram_tensor("x", [B_LOC, T, C, H, W], FP32, kind="ExternalInput").ap()
    o_d = nc.dram_tensor("out", [B_LOC, T, C, H, W], U8, kind="ExternalOutput").ap()
    with tile.TileContext(nc) as tc:
        _emit(tc, x_d, o_d)
    nc.compile()
    _NC_CACHE["nc"] = nc
    return nc


def make_in_maps(x: np.ndarray) -> list[dict[str, np.ndarray]]:
    assert x.shape == (B, T, C, H, W) and x.dtype == np.float32, (x.shape, x.dtype)
    return [
        {"x": np.ascontiguousarray(x[i * B_LOC : (i + 1) * B_LOC])}
        for i in range(N_CORES)
    ]


def kernel(x: np.ndarray) -> np.ndarray:
    x = np.asarray(x, dtype=np.float32)
    nc = build_nc()
    res = run_bass_kernel_spmd(nc, make_in_maps(x), list(range(N_CORES)))
    out_u8 = np.concatenate([r["out"] for r in res.results], axis=0)
    return out_u8.astype(np.float32)


# revision 8
# speedup vs baseline: 3.7233x; 3.7233x over previous
"""LIF spiking-neuron kernel for Trainium2 (Bass/Tile), 8-core SPMD.

Problem: x [B=32, T=8, C=128, H=32, W=32] f32.  Per (b,c,h,w) neuron,
sequential over T:
    mem = mem*TAU + x_t;  spike = (mem - 1 > 0);  mem = 0 if spike
TAU = 0.5, THRESH = 1.0.

Sharding: batch dim B=32 split 4-per-core across 8 NeuronCores; the
recurrence is along T only, so there is no communication.

Per-core algorithm (bit-exact vs the fp32 reference):
  TAU = 0.5 is a power of two, so rescale the state M_t = 2^t * m_t.
  The decay becomes a pure add:  M_t = M_{t-1} + 2^t * x_t  (the 2^t
  prescale of x is exact in fp32, and power-of-2 scaling commutes with
  fp rounding, so every M_t is bit-exactly 2^t * m_t).
  spike_t = (M_t > 2^t)  <=>  (m_t > 1)  <=>  reference's (m_t - 1 > 0).

  The whole state update (previous step's reset + prescale + accumulate)
  is ONE fused custom-DVE op per step:
      M_t = select(M_{t-1} > 2^(t-1), 0, M_{t-1}) + x_t * 2^t
  so the spike computation is only an output tap, off the recurrence
  critical path.  Spikes are written to DRAM as uint8 (4x less output
  DMA traffic than f32) and upcast to f32 on the host.

Engine split per (b,t) tile of [C=128 partitions, H*W*PAIR=2048]:
  DVE:  fused state update (14 ops/core) -- the only recurrence work
  ACT:  chain-0 spikes: Sign(M - 2^t) -> u8 (saturating cast maps -1 to
        0, verified on HW), plus chain-1 input DMA triggers and chain-0
        output DMA triggers
  GPS:  chain-1 spikes: (M > 2^t) -> u8 via tensor_single_scalar
  SP :  chain-0 input DMA triggers, chain-1 output DMA triggers
"""

import re

import numpy as np

from concourse import bacc, bass, mybir, tile
from concourse import dve_ops
from concourse.alu_op_type import AluOpType
from concourse.bass_utils import run_bass_kernel_spmd
from concourse.dve_spec import Spec, Src0, Src1, C0, C1, Zero, select

# Full-problem shape (hardcoded per harness contract).
B, T, C, H, W = 32, 8, 128, 32, 32
N_CORES = 8
B_LOC = B // N_CORES          # 4 batches per core
F = H * W                     # 1024 free elements per (b, t, c)
FP32 = mybir.dt.float32
U8 = mybir.dt.uint8

PAIR = 2                      # batches fused per tile
G = B_LOC // PAIR             # chain groups per core
FW = PAIR * F                 # 2048 free elements per tile

_NC_CACHE = {}


def _register_lif_op():
    """Register the fused LIF state-update custom-DVE op (idempotent).

    out = select(in1 > s1, 0, in1) + in0 * s0
        = (previous step's hard reset) + (2^t-prescaled input)
    """
    name = "LIF_STEP_ANT"
    if name in dve_ops._SUB_OPCODE_FOR_NAME:
        return next(op for op in dve_ops.OPS if op.name == name)
    body = select(Src1 > C1, Zero, Src1) + Src0 * C0
    op = dve_ops.DveOp(
        name,
        Spec(
            body=body,
            reference=lambda in0, in1, s0, s1, imm2: (
                np.where(in1 > s1, np.float32(0.0), in1) + in0 * s0
            ).astype(np.float32),
        ),
        subdim=False,
        uops_sha={},
    )
    dve_ops.OPS.append(op)
    dve_ops.CUSTOM_DVE_SPECS[name] = op.spec
    dve_ops._SUB_OPCODE_FOR_NAME[name] = (
        dve_ops._CUSTOM_DVE_ROW_BASE + len(dve_ops.OPS) - 1
    )
    # Pin uops_sha to whatever lower() produces in this process.
    for ver in ("v3", "v4"):
        try:
            op.compile(ver)
        except ValueError as e:
            m = re.search(r'"%s"\]="([0-9a-f]{16})"' % ver, str(e))
            if not m:
                raise
            op.uops_sha[ver] = m.group(1)
            dve_ops._COMPILE_CACHE.pop((name, ver), None)
            op.compile(ver)
    return op


LIF_STEP = _register_lif_op()


def _emit(tc, x_d, o_d):
    nc = tc.nc

    # per (group, t) DRAM view: [c, pair, h*w] -- 2 batches fused per tile
    def dram3(ap, g, t):
        return ap[g * PAIR : (g + 1) * PAIR, t].rearrange("p c h w -> c p (h w)")

    def as3(tile_ap):
        return tile_ap.rearrange("c (p f) -> c p f", p=PAIR)

    with (
        tc.tile_pool(name="xp", bufs=(T - 1) * G) as xp,
        tc.tile_pool(name="sp", bufs=6) as sp,
        tc.tile_pool(name="mp", bufs=6) as mp,
        tc.tile_pool(name="bp", bufs=1) as bp,
    ):
        # per-t [128,1] bias columns holding -2^t for the ACT Sign compare.
        # NOTE: keep GpSimd completely idle — its software ops are ~15x
        # slower than DVE and it shares SBUF ports with DVE (running
        # anything there starves the recurrence chain).
        biases = []
        for t in range(T):
            bt = bp.tile([C, 1], FP32, name=f"bias{t}")
            nc.vector.memset(bt, -float(2.0**t))
            biases.append(bt)

        # --- all input DMAs issued up front, t-major, SP/ACT split by chain.
        # t=0 lands directly in the chain's first membrane tile (M_0 = x_0).
        ms = {}
        for g in range(G):
            m0 = mp.tile([C, FW], FP32, name="mt")
            eng = nc.sync if g == 0 else nc.scalar
            eng.dma_start(out=as3(m0), in_=dram3(x_d, g, 0))
            ms[g] = m0
        xs = {}
        for t in range(1, T):
            for g in range(G):
                xt = xp.tile([C, FW], FP32)
                eng = nc.sync if g == 0 else nc.scalar
                eng.dma_start(out=as3(xt), in_=dram3(x_d, g, t))
                xs[(t, g)] = xt

        # --- recurrence (DVE) + spike taps (ACT / GPS) + u8 output DMAs
        for t in range(T):
            th = float(2.0**t)
            for g in range(G):
                if t > 0:
                    m_new = mp.tile([C, FW], FP32, name="mt")
                    nc.vector._custom_dve(
                        LIF_STEP,
                        out=m_new,
                        in0=xs[(t, g)],
                        in1=ms[g],
                        s0=th,
                        s1=th / 2.0,
                    )
                    ms[g] = m_new
                m = ms[g]
                s = sp.tile([C, FW], U8)
                # ACT: u8 cast of Sign(M - 2^t) saturates -1 -> 0, so the
                # spike is a single activation op for both chains.
                nc.scalar.activation(
                    s, m, mybir.ActivationFunctionType.Sign, bias=biases[t]
                )
                eng = nc.scalar if g == 0 else nc.sync
                eng.dma_start(out=dram3(o_d, g, t), in_=as3(s))


def build_nc():
    """Build + compile the per-core Bass program (cached)."""
    if "nc" in _NC_CACHE:
        return _NC_CACHE["nc"]
    nc = bacc.Bacc(
        "TRN2",
        target_bir_lowering=False,
        debug=False,
        enable_asserts=False,
        num_devices=N_CORES,
    )
    x_d = nc.dram_tensor("x", [B_LOC, T, C, H, W], FP32, kind="ExternalInput").ap()
    o_d = nc.dram_tensor("out", [B_LOC, T, C, H, W], U8, kind="ExternalOutput").ap()
    with tile.TileContext(nc) as tc:
        _emit(tc, x_d, o_d)
    nc.compile()
    _NC_CACHE["nc"] = nc
    return nc


def make_in_maps(x: np.ndarray) -> list[dict[str, np.ndarray]]:
    assert x.shape == (B, T, C, H, W) and x.dtype == np.float32, (x.shape, x.dtype)
    return [
        {"x": np.ascontiguousarray(x[i * B_LOC : (i + 1) * B_LOC])}
        for i in range(N_CORES)
    ]


def kernel(x: np.ndarray) -> np.ndarray:
    x = np.asarray(x, dtype=np.float32)
    nc = build_nc()
    res = run_bass_kernel_spmd(nc, make_in_maps(x), list(range(N_CORES)))
    out_u8 = np.concatenate([r["out"] for r in res.results], axis=0)
    return out_u8.astype(np.float32)
